# Initial kernel scaffold
#
"""Trainium2 Bass kernel for batched per-sample expert matmul (MoE routing).

Computes y[n, i] = relu(b[idxs[n], i] + sum_o w[idxs[n], i, o] * x[n, o])
for x (8192, 256), idxs (8192,), w (64, 256, 256), b (64, 256).

Strategy
--------
Host side (numpy, cheap):
  * Stable-sort all 8192 samples by expert id, shard the *sorted* batch
    contiguously across the 8 cores (1024 samples each). Each core's
    samples then span only a handful of contiguous experts, so the
    per-core weight traffic is ~3 MB instead of the full 16 MB table.
  * Cut each core's samples into segments of <= 128 samples, one expert
    per segment, padded so every core runs the same NSEG segments
    (SPMD: one program, per-core data).
  * Pre-gather, per segment: the expert's weight matrix laid out for the
    PE (contraction dim on partitions) and its bias row.  Pre-transpose
    the sorted x so the contraction dim is on partitions.

Device side (one static Tile program, identical on all 8 cores):
  for each segment s:
      psum[m, i]  = 1 * b[i]                       (K=1 bias matmul)
      psum[m, i] += sum_p xT0[p, m] * w0[p, i]     (K-chunk 0)
      psum[m, i] += sum_p xT1[p, m] * w1[p, i]     (K-chunk 1)
      y[m, i]     = relu(psum[m, i])               (ACT, PSUM -> SBUF)

  This walrus build allows only ONE semaphore wait on lowered matmul
  (LDWEIGHTS+MATMUL) and HWDGE direct-DMA instructions, so the program
  is shaped so nothing ever needs two: <= 8 DMAs total (no DMA-lane
  recycling), each weight batch in its own SBUF slot (no WAW waits),
  the bias matmul opens each PSUM group (operands resident; carries
  only the PSUM-release wait), K-chunk matmuls carry only the
  weight-DMA wait (x residency absorbed by a prologue dummy matmul),
  and ones/bias share one DMA.

Host side: scatter segment rows back to the original sample order.
Inputs with pathological expert skew can exceed the per-pass segment
budget; those run the same program over multiple passes.
"""

import os

import numpy as np

import concourse.bacc as bacc
import concourse.bass as bass
import concourse.mybir as mybir
import concourse.tile as tile
from concourse.bass_utils import run_bass_kernel_spmd

N_CORES = 8
P = 128          # SBUF/PSUM partitions
F = 256          # feature dim (in_features == out_features == 256)
SEG = 128        # samples per segment (== max PSUM partition dim)
OGS = 4          # segments per output-DMA batch
MAX_NSEG = 32    # per-pass segment budget (SBUF residency bound)


def _batches(n, sizes, rest):
    """Split range(n) into batches: explicit `sizes` first, then `rest`-sized."""
    out = []
    lo = 0
    i = 0
    while lo < n:
        sz = sizes[i] if i < len(sizes) else rest
        i += 1
        hi = min(n, lo + sz)
        out.append((lo, hi))
        lo = hi
    return out


def _stream_batches(n):
    """Batch plans for the weight and x streams over n segments.

    Small head batches (compute starts early), large middle ones (amortize
    the per-DMA fixed cost), a tiny tail batch (minimal compute left after
    the stream ends). x-batch starts are staggered against w-batch starts
    so no K-chunk matmul needs both an x- and a w-DMA wait.
    """
    if n <= 4:
        wbat = _batches(n, [1], 2)
    else:
        mid = n - 8
        wsz = [1, 2] + ([4] * (mid // 4)) + ([mid % 4] if mid % 4 else []) + [4, 1]
        wbat = _batches(n, wsz, 4)
    wstarts = {lo for lo, _ in wbat}
    starts = [0]
    for lo, _ in _batches(n, [2, 5], 6)[1:]:
        while lo in wstarts or lo <= starts[-1]:
            lo += 1
        if lo < n:
            starts.append(lo)
    xbat = [
        (starts[i], starts[i + 1] if i + 1 < len(starts) else n)
        for i in range(len(starts))
    ]
    return wbat, xbat
MM_DT = (
    mybir.dt.float32
    if os.environ.get("KBENCH_MM_DT", "float32r") == "float32"
    else mybir.dt.float32r
)  # matmul operand dtype; float32r streams 4x faster at moving dim >= 256

# Set by the last kernel() call when KBENCH_TRACE=1 (used by test.py only).
LAST_EXEC_TIME_NS = None
LAST_TRACE = None


def _build_schedule(idxs: np.ndarray):
    """Sort samples by expert, shard, and cut per-core single-expert segments."""
    B = idxs.shape[0]
    S = B // N_CORES
    order = np.argsort(idxs, kind="stable")
    sidx = idxs[order]
    per_core = []
    for c in range(N_CORES):
        e = sidx[c * S:(c + 1) * S]
        # run-length encode the (sorted) expert ids of this core's chunk
        segs = []  # (expert, local_start, count), count <= SEG
        i = 0
        while i < S:
            j = i
            while j < S and e[j] == e[i]:
                j += 1
            k = i
            while k < j:
                cnt = min(SEG, j - k)
                segs.append((int(e[i]), k, cnt))
                k += cnt
            i = j
        per_core.append(segs)
    return order, per_core


def _build_program(nseg: int):
    nc = bacc.Bacc(
        "TRN2", target_bir_lowering=False, debug=False, num_devices=N_CORES
    )
    npad = nseg * SEG
    xt_d = nc.dram_tensor("xt", [P, 2, npad], MM_DT, kind="ExternalInput").ap()
    w_d = nc.dram_tensor("wseg", [P, nseg, 2 * F], MM_DT, kind="ExternalInput").ap()
    b_d = nc.dram_tensor(
        "bconst", [1, P + nseg * F], MM_DT, kind="ExternalInput"
    ).ap()
    y_d = nc.dram_tensor(
        "y", [P, nseg, F], mybir.dt.float32, kind="ExternalOutput"
    ).ap()

    f32 = mybir.dt.float32
    relu = mybir.ActivationFunctionType.Relu

    wbat, xbat = _stream_batches(nseg)
    obat = _batches(nseg, [], OGS)

    with tile.TileContext(nc) as tc:
        with (
            tc.tile_pool(name="const", bufs=1) as const,
            tc.tile_pool(name="w", bufs=1) as wpool,
            tc.tile_pool(name="yout", bufs=1) as ypool,
            tc.tile_pool(name="ps", bufs=3, space="PSUM") as pspool,
            tc.tile_pool(name="scr", bufs=1, space="PSUM") as scrpool,
        ):
            # bconst rides the otherwise-idle HWDGE ring; the x/w streams go
            # through the single SWDGE queue (gpsimd): FIFO delivery in issue
            # order at full line rate, one completion semaphore per batch ->
            # a just-in-time pipeline.  (Independent HWDGE queues share SDMA
            # bandwidth round-robin, which delays the earliest transfer.)
            bc = const.tile([1, P + nseg * F], MM_DT, tag="bconst")
            nc.sync.dma_start(bc[:], b_d[:])

            xts = {}

            def load_x_batch(b):
                lo, hi = xbat[b]
                t = const.tile([P, 2 * (hi - lo) * SEG], MM_DT, tag=f"xt{b}")
                xts[b] = t
                nc.gpsimd.dma_start(
                    t[:].rearrange("p (c n) -> p c n", c=2),
                    xt_d[:, :, lo * SEG:hi * SEG],
                )

            wts = {}

            def load_w_batch(g):
                lo, hi = wbat[g]
                t = wpool.tile([P, (hi - lo) * 2 * F], MM_DT, tag=f"w{g}")
                wts[g] = t
                nc.gpsimd.dma_start(
                    t[:], w_d[:, lo:hi, :].rearrange("p g f -> p (g f)")
                )

            ones = bc[:, 0:P]

            seg2x = {}
            for b, (lo, hi) in enumerate(xbat):
                for s in range(lo, hi):
                    seg2x[s] = b
            seg2w = {}
            for g, (lo, hi) in enumerate(wbat):
                for s in range(lo, hi):
                    seg2w[s] = g

            def xchunk(s, c):
                b = seg2x[s]
                lo, hi = xbat[b]
                base = (c * (hi - lo) + (s - lo)) * SEG
                return xts[b][:, base:base + SEG]

            def wchunk(s, c):
                g = seg2w[s]
                lo, hi = wbat[g]
                base = (2 * (s - lo) + c) * F
                return wts[g][:, base:base + F]

            # need-order emission on the SWDGE queue
            nxt_x = 0
            for g in range(len(wbat)):
                while nxt_x < len(xbat) and xbat[nxt_x][0] <= wbat[g][0]:
                    load_x_batch(nxt_x)
                    nxt_x += 1
                load_w_batch(g)
            while nxt_x < len(xbat):
                load_x_batch(nxt_x)
                nxt_x += 1

            scr = scrpool.tile([2, 2], f32)
            yt = None
            # Process segments in pairs sharing one full PSUM bank: a single
            # N=512 bias matmul covers both, and one ACT relu drains both.
            pairs = _batches(nseg, [], 2)
            for plo, phi in pairs:
                pw = (phi - plo) * F
                if plo % OGS == 0:
                    yt = ypool.tile([P, OGS * F], f32, tag=f"y{plo // OGS}")
                if plo == 0:
                    # Absorb x batch 0's DMA wait into the PE's clock so
                    # segment 0's K-chunk matmul only needs the w-DMA wait.
                    xb = xts[0]
                    nc.tensor.matmul(
                        scr[:], xb[:, 0:2], xb[:, 0:2], start=True, stop=True
                    )
                ps = pspool.tile([P, 2 * F], f32)
                nc.tensor.matmul(
                    ps[:, 0:pw],
                    ones,
                    bc[:, P + plo * F:P + phi * F],
                    start=True,
                    stop=False,
                )
                for s in range(plo, phi):
                    o = (s - plo) * F
                    nc.tensor.matmul(
                        ps[:, o:o + F], xchunk(s, 0), wchunk(s, 0),
                        start=False, stop=False,
                    )
                    nc.tensor.matmul(
                        ps[:, o:o + F], xchunk(s, 1), wchunk(s, 1),
                        start=False, stop=(s == phi - 1),
                    )
                j = plo % OGS
                # relu on DVE: keeps ACT (and its 1.3us table-load preamble)
                # out of the kernel entirely.
                nc.vector.tensor_scalar_max(
                    yt[:, j * F:j * F + pw], ps[:, 0:pw], 0.0
                )
                if phi % OGS == 0 or phi == nseg:
                    lo = (plo // OGS) * OGS
                    nc.sync.dma_start(
                        y_d[:, lo:phi, :].rearrange("p g f -> p (g f)"),
                        yt[:, 0:(phi - lo) * F],
                    )
    nc.compile()
    return nc


def kernel(x: np.ndarray, idxs: np.ndarray, w: np.ndarray, b: np.ndarray) -> np.ndarray:
    global LAST_EXEC_TIME_NS, LAST_TRACE
    x = np.ascontiguousarray(x, dtype=np.float32)
    w = np.ascontiguousarray(w, dtype=np.float32)
    b = np.ascontiguousarray(b, dtype=np.float32)
    idxs_np = np.asarray(idxs).astype(np.int64)

    B = x.shape[0]
    S = B // N_CORES
    order, per_core = _build_schedule(idxs_np)

    # Split each core's segment list into passes of <= MAX_NSEG segments.
    npass = max(1, (max(len(s) for s in per_core) + MAX_NSEG - 1) // MAX_NSEG)
    if npass == 1:
        nseg = max(2, max(len(s) for s in per_core))
    else:
        nseg = MAX_NSEG
    npad = nseg * SEG

    # Per-expert weight blocks in PE layout:
    # wprep[e, p, c*F + i] = w[e, i, c*P + p]  (c = contraction chunk 0/1)
    wprep = np.ascontiguousarray(
        w.transpose(0, 2, 1)           # (e, o, i)
        .reshape(64, 2, P, F)          # (e, c, p, i)
        .transpose(0, 2, 1, 3)         # (e, p, c, i)
        .reshape(64, P, 2 * F)
    )

    nc = _build_program(nseg)
    trace = bool(os.environ.get("KBENCH_TRACE"))

    y = np.empty((B, F), dtype=np.float32)
    for pi in range(npass):
        in_maps = []
        for c in range(N_CORES):
            sel = order[c * S:(c + 1) * S]
            segs = per_core[c][pi * MAX_NSEG:(pi + 1) * MAX_NSEG]
            xpad = np.zeros((npad, F), dtype=np.float32)
            eids = np.zeros(nseg, dtype=np.int64)
            for s, (e, k0, cnt) in enumerate(segs):
                xpad[s * SEG:s * SEG + cnt] = x[sel[k0:k0 + cnt]]
                eids[s] = e
            # xt[p, c, n] = xpad[n, c*P + p]
            xt = np.ascontiguousarray(
                xpad.T.reshape(2, P, npad).transpose(1, 0, 2)
            )
            wseg = np.ascontiguousarray(
                wprep[eids].transpose(1, 0, 2)
            )  # (P, nseg, 2F)
            bconst = np.concatenate(
                [np.ones(P, dtype=np.float32), b[eids].reshape(nseg * F)]
            ).reshape(1, P + nseg * F)
            in_maps.append({"xt": xt, "wseg": wseg, "bconst": bconst})

        res = run_bass_kernel_spmd(
            nc, in_maps, core_ids=list(range(N_CORES)), trace=trace
        )
        LAST_EXEC_TIME_NS = res.exec_time_ns
        LAST_TRACE = res.instructions_and_trace

        for c in range(N_CORES):
            sel = order[c * S:(c + 1) * S]
            segs = per_core[c][pi * MAX_NSEG:(pi + 1) * MAX_NSEG]
            ypad = res.results[c]["y"].transpose(1, 0, 2).reshape(npad, F)
            for s, (e, k0, cnt) in enumerate(segs):
                y[sel[k0:k0 + cnt]] = ypad[s * SEG:s * SEG + cnt]
    return y



# revision 15
# speedup vs baseline: 1.3918x; 1.3918x over previous
"""Trainium2 Bass kernel for batched per-sample expert matmul (MoE routing).

Computes y[n, i] = relu(b[idxs[n], i] + sum_o w[idxs[n], i, o] * x[n, o])
for x (8192, 256), idxs (8192,), w (64, 256, 256), b (64, 256).

Strategy (expert-parallel, bf16 traffic)
-----------------------------------------
The whole problem is DMA-roofline bound (~330-360 GB/s aggregate per
core), so the kernel is shaped around minimizing HBM bytes:

Host side (numpy, cheap):
  * Route tokens by expert and assign each expert to exactly ONE core
    (greedy balance on token counts) -> every weight matrix crosses HBM
    exactly once, on one core: 1.0 MB/core of bf16 weights instead of
    the 16 MB replicated table.
  * All matmul traffic is cast to bf16 (inputs, weights, outputs). The
    error budget (absmax-relative 2e-2) leaves ~10x margin over bf16's
    ~1e-3.
  * Each core gets NSLOT expert slots; slot j is padded to a uniform
    per-rank token capacity cap[j] (slots sorted by count, so rank
    capacities hug the actual distribution).  One DRAM input tensor per
    core packs [w | xT] per slot in consumption order.

Device side (one static Tile program, identical on all 8 cores):
  Output features live on PSUM partitions, tokens on the free dim, so
  one expert = one PSUM bank and the bias is a per-partition scalar:
    ps[:, 0:C]       = wT00 @ x0 + wT10 @ x1      (i-chunk 0)
    ps[:, 256:256+C] = wT01 @ x0 + wT11 @ x1      (i-chunk 1)
    y0 = max(ps[:, 0:C] + bias0, 0)      (DVE, per-partition bias)
    y1 = relu(ps[:, 256:..] * 1 + bias1) (ACT, runs parallel to DVE)
  No bias matmuls, no weight duplication; 8 slots = 8 PSUM banks, so no
  PSUM reuse waits.  The in-stream rides the single SWDGE (gpsimd)
  queue in need order; outputs leave via the Sync HWDGE ring.  Dummy
  DVE/ACT ops absorb the bias-DMA wait so every matmul and drain
  carries at most one semaphore wait (walrus constraint).

Host side: scatter the per-slot token blocks back to sample order.
Pathological expert skew (>256 tokens per chunk slot or >8 slots per
core) falls back to extra passes of the same program shape.
"""

import os

import numpy as np

import concourse.bacc as bacc
import concourse.bass as bass
import concourse.mybir as mybir
import concourse.tile as tile
from concourse.bass_utils import run_bass_kernel_spmd

try:
    import ml_dtypes

    BF16 = ml_dtypes.bfloat16
except ImportError:  # pragma: no cover
    BF16 = np.dtype("bfloat16")

N_CORES = 8
P = 128
F = 256
CAP_MAX = 256    # tokens per slot (2 i-chunks of <=256 f32 fill one PSUM bank)
NSLOT = 8        # expert slots per core per pass == PSUM banks

# Set by the last kernel() call when KBENCH_TRACE=1 (used by test.py only).
LAST_EXEC_TIME_NS = None
LAST_TRACE = None

_PROGRAM_CACHE = {}


def _build_schedule(idxs: np.ndarray):
    """Assign expert chunks (<=CAP_MAX tokens) to cores, balanced by count.

    Returns (passes, order) where passes is a list of scheduling passes;
    each pass is a list of per-core slot lists [(expert, tok_array), ...]
    sorted by descending token count.
    """
    toks_by_e = [np.nonzero(idxs == e)[0] for e in range(64)]
    chunks = []
    for e, toks in enumerate(toks_by_e):
        for k in range(0, len(toks), CAP_MAX):
            chunks.append((e, toks[k:k + CAP_MAX]))
    chunks.sort(key=lambda c: -len(c[1]))

    npass = max(1, -(-len(chunks) // (N_CORES * NSLOT)))
    cores = [[] for _ in range(N_CORES * npass)]
    load = [0] * (N_CORES * npass)
    for e, toks in chunks:
        cand = min(
            (i for i in range(len(cores)) if len(cores[i]) < NSLOT),
            key=lambda i: load[i],
        )
        cores[cand].append((e, toks))
        load[cand] += len(toks)
    for sl in cores:
        sl.sort(key=lambda c: -len(c[1]))
    return [cores[p * N_CORES:(p + 1) * N_CORES] for p in range(npass)]


def _build_program(caps: tuple):
    nslot = len(caps)
    S = sum(caps)
    xoff = [0]
    for c in caps:
        xoff.append(xoff[-1] + c)

    int8_w = os.environ.get("KINT8", "1") == "1"

    nc = bacc.Bacc(
        "TRN2", target_bir_lowering=False, debug=False, num_devices=N_CORES
    )
    bf16 = mybir.dt.bfloat16
    f32 = mybir.dt.float32
    bias_d = nc.dram_tensor("bias", [P, 2 * nslot], f32, kind="ExternalInput").ap()
    y_d = nc.dram_tensor("y", [P, 2 * S], bf16, kind="ExternalOutput").ap()

    relu = mybir.ActivationFunctionType.Relu
    add = mybir.AluOpType.add
    amax = mybir.AluOpType.max

    # in-stream batches over whole slots: small head batch to prime the
    # pipeline.  The int8 path pays two DMAs (w, x) per batch, so it uses
    # fewer, bigger batches to keep SWDGE descriptor generation (~0.7us per
    # DMA) below the wire time.
    if nslot <= 2:
        groups = [[j] for j in range(nslot)]
    elif int8_w:
        mid = (nslot + 1) // 2
        groups = [[0], list(range(1, mid + 1)), list(range(mid + 1, nslot))]
        groups = [g for g in groups if g]
    else:
        groups = [[0]]
        mid = list(range(1, nslot - 1))
        groups += [mid[i:i + 3] for i in range(0, len(mid), 3)]
        groups += [[nslot - 1]]

    inq = nc.gpsimd
    outq = nc.sync

    with tile.TileContext(nc) as tc:
        with (
            tc.tile_pool(name="const", bufs=1) as const,
            tc.tile_pool(name="inb", bufs=1) as inpool,
            tc.tile_pool(name="yout", bufs=1) as ypool,
            tc.tile_pool(name="ps", bufs=8, space="PSUM") as pspool,
        ):
            bt = const.tile([P, 2 * nslot], f32, tag="bias")
            outq.dma_start(bt[:], bias_d[:])

            tiles = {}
            xtiles = {}
            if int8_w:
                # w ships as int8 (values in [-127,127], exact in bf16) and
                # the SWDGE casts to bf16 in flight; the per-contraction-row
                # quant scales are folded into x on the host.  w and x
                # alternate on the one gpsimd queue in need order, so a slot
                # still needs only a single (later-threshold) wait.
                w_d = nc.dram_tensor(
                    "wt", [P, nslot * 4 * P], mybir.dt.int8, kind="ExternalInput"
                ).ap()
                x_d = nc.dram_tensor(
                    "xt", [P, 2 * S], bf16, kind="ExternalInput"
                ).ap()
                for g, slots in enumerate(groups):
                    lo, hi = slots[0], slots[-1] + 1
                    t = inpool.tile([P, (hi - lo) * 4 * P], bf16, tag=f"w{g}")
                    for j in slots:
                        tiles[j] = (t, (j - lo) * 4 * P)
                    inq.dma_start(t[:], w_d[:, lo * 4 * P:hi * 4 * P])
                    xt = inpool.tile(
                        [P, 2 * (xoff[hi] - xoff[lo])], bf16, tag=f"x{g}"
                    )
                    for j in slots:
                        xtiles[j] = (xt, 2 * (xoff[j] - xoff[lo]))
                    inq.dma_start(xt[:], x_d[:, 2 * xoff[lo]:2 * xoff[hi]])
            else:
                NCOL = nslot * 4 * P + 2 * S
                in_d = nc.dram_tensor(
                    "inp", [P, NCOL], bf16, kind="ExternalInput"
                ).ap()

                def slot_col(j):
                    return j * 4 * P + 2 * xoff[j]

                for g, slots in enumerate(groups):
                    lo, hi = slots[0], slots[-1] + 1
                    a, b = slot_col(lo), slot_col(hi) if hi < nslot else NCOL
                    t = inpool.tile([P, b - a], bf16, tag=f"in{g}")
                    for j in slots:
                        tiles[j] = (t, slot_col(j) - a)
                        xtiles[j] = (t, slot_col(j) - a + 4 * P)
                    inq.dma_start(t[:], in_d[:, a:b])

            def wv(j, c0, c1):
                t, base = tiles[j]
                o = base + (c0 * 2 + c1) * P
                return t[:, o:o + P]

            def xv(j, c0):
                t, base = xtiles[j]
                o = base + c0 * caps[j]
                return t[:, o:o + caps[j]]

            yt0 = ypool.tile([P, S], bf16, tag="y0")
            yt1 = ypool.tile([P, S], bf16, tag="y1")
            scr0 = const.tile([P, 1], f32, tag="scr0")
            scr1 = const.tile([P, 1], f32, tag="scr1")
            # Absorb the bias-DMA wait (and ACT's one-time relu table load)
            # off the critical path so the real drains carry only the PE
            # semaphore wait.
            nc.vector.tensor_scalar(scr0[:], bt[:, 0:1], 0.0, None, add)
            nc.scalar.activation(scr1[:], bt[:, 0:1], relu)

            owave = [0]
            flush_at = {nslot - 1}
            if nslot > 2:
                flush_at.add(nslot - 2)
            if nslot > 5:
                flush_at.add(nslot // 2)
            for j in range(nslot):
                C = caps[j]
                ps = pspool.tile([P, 2 * F], f32)
                # One accumulation group per PSUM bank: start=True zeroes the
                # WHOLE bank, so only the first matmul opens it and only the
                # last one closes it.
                nc.tensor.matmul(
                    ps[:, 0:C], wv(j, 0, 0), xv(j, 0), start=True, stop=False
                )
                nc.tensor.matmul(
                    ps[:, F:F + C], wv(j, 0, 1), xv(j, 0), start=False, stop=False
                )
                nc.tensor.matmul(
                    ps[:, 0:C], wv(j, 1, 0), xv(j, 1), start=False, stop=False
                )
                nc.tensor.matmul(
                    ps[:, F:F + C], wv(j, 1, 1), xv(j, 1), start=False, stop=True
                )
                o = xoff[j]
                nc.vector.tensor_scalar(
                    yt0[:, o:o + C], ps[:, 0:C], bt[:, 2 * j:2 * j + 1], 0.0,
                    add, amax,
                )
                nc.scalar.activation(
                    yt1[:, o:o + C], ps[:, F:F + C], relu,
                    bias=bt[:, 2 * j + 1:2 * j + 2],
                )
                # Flush outputs in waves; the last wave is just the smallest
                # slot so the post-stream tail stays short.
                if j in flush_at:
                    lo, hi = owave[0], xoff[j + 1]
                    owave = [hi]
                    outq.dma_start(y_d[:, lo:hi], yt0[:, lo:hi])
                    outq.dma_start(y_d[:, S + lo:S + hi], yt1[:, lo:hi])
    nc.compile()
    return nc


def kernel(x: np.ndarray, idxs: np.ndarray, w: np.ndarray, b: np.ndarray) -> np.ndarray:
    global LAST_EXEC_TIME_NS, LAST_TRACE
    x = np.ascontiguousarray(x, dtype=np.float32)
    w = np.ascontiguousarray(w, dtype=np.float32)
    b = np.ascontiguousarray(b, dtype=np.float32)
    idxs_np = np.asarray(idxs).astype(np.int64)
    B = x.shape[0]

    int8_w = os.environ.get("KINT8", "1") == "1"
    if int8_w:
        # Symmetric int8 per (expert, contraction row) o; scales are folded
        # into x per slot on the host, so the device sees plain bf16 math.
        wscale = np.abs(w).max(axis=1) / 127.0          # (64, 256)
        wscale = np.maximum(wscale, 1e-30)
        wq = np.round(w / wscale[:, None, :]).clip(-127, 127).astype(np.int8)
        wblk = np.ascontiguousarray(
            wq.reshape(64, 2, P, 2, P)     # (e, c1, m, c0, p)
            .transpose(0, 4, 3, 1, 2)      # (e, p, c0, c1, m)
            .reshape(64, P, 4 * P)
        )
    else:
        x16 = x.astype(BF16)
        # Per-expert weight blocks in PE layout:
        # wblk[e, p, (c0*2+c1)*128 + m] = w[e, c1*128+m, c0*128+p]
        wblk = np.ascontiguousarray(
            w.reshape(64, 2, P, 2, P)          # (e, c1, m, c0, p)
            .transpose(0, 4, 3, 1, 2)          # (e, p, c0, c1, m)
            .reshape(64, P, 4 * P)
            .astype(BF16)
        )

    passes = _build_schedule(idxs_np)
    trace = bool(os.environ.get("KBENCH_TRACE"))
    y = np.empty((B, F), dtype=np.float32)

    for cores in passes:
        nslot = max(1, max(len(sl) for sl in cores))
        caps = tuple(
            max(4, -4 * (-max(
                (len(sl[j][1]) if j < len(sl) else 0) for sl in cores
            ) // 4))
            for j in range(nslot)
        )
        S = sum(caps)
        xoff = np.concatenate([[0], np.cumsum(caps)]).astype(int)
        NCOL = nslot * 4 * P + 2 * S

        key = caps
        if key not in _PROGRAM_CACHE:
            _PROGRAM_CACHE[key] = _build_program(caps)
        nc = _PROGRAM_CACHE[key]

        in_maps = []
        for sl in cores:
            bias = np.zeros((P, 2 * nslot), dtype=np.float32)
            if int8_w:
                wt = np.zeros((P, nslot * 4 * P), dtype=np.int8)
                xt_full = np.zeros((P, 2 * S), dtype=BF16)
            else:
                inp = np.zeros((P, NCOL), dtype=BF16)
            for j, (e, toks) in enumerate(sl):
                n = len(toks)
                if int8_w:
                    wt[:, j * 4 * P:(j + 1) * 4 * P] = wblk[e]
                    xs = (x[toks] * wscale[e]).astype(BF16)
                    xt = xs.T.reshape(2, P, n).transpose(1, 0, 2)
                    xcols = xt_full[:, 2 * xoff[j]:2 * xoff[j + 1]].reshape(
                        P, 2, caps[j]
                    )
                else:
                    col = j * 4 * P + 2 * xoff[j]
                    inp[:, col:col + 4 * P] = wblk[e]
                    # xT[p, c0, t] = x[tok_t, c0*128 + p]
                    xt = x16[toks].T.reshape(2, P, n).transpose(1, 0, 2)
                    xcols = inp[:, col + 4 * P:col + 4 * P + 2 * caps[j]].reshape(
                        P, 2, caps[j]
                    )
                xcols[:, :, :n] = xt
                bias[:, 2 * j] = b[e, 0:P]
                bias[:, 2 * j + 1] = b[e, P:2 * P]
            if int8_w:
                in_maps.append({"wt": wt, "xt": xt_full, "bias": bias})
            else:
                in_maps.append({"inp": inp, "bias": bias})

        res = run_bass_kernel_spmd(
            nc, in_maps, core_ids=list(range(N_CORES)), trace=trace
        )
        LAST_EXEC_TIME_NS = res.exec_time_ns
        LAST_TRACE = res.instructions_and_trace

        for c, sl in enumerate(cores):
            yc = np.asarray(res.results[c]["y"]).reshape(P, 2, S)
            for j, (e, toks) in enumerate(sl):
                n = len(toks)
                o = xoff[j]
                y[toks] = (
                    yc[:, :, o:o + n].transpose(2, 1, 0).reshape(n, F)
                )
    return y


# revision 17
# speedup vs baseline: 1.4254x; 1.0242x over previous
"""Trainium2 Bass kernel for batched per-sample expert matmul (MoE routing).

Computes y[n, i] = relu(b[idxs[n], i] + sum_o w[idxs[n], i, o] * x[n, o])
for x (8192, 256), idxs (8192,), w (64, 256, 256), b (64, 256).

Strategy (expert-parallel, bf16 traffic)
-----------------------------------------
The whole problem is DMA-roofline bound (~330-360 GB/s aggregate per
core), so the kernel is shaped around minimizing HBM bytes:

Host side (numpy, cheap):
  * Route tokens by expert and assign each expert to exactly ONE core
    (greedy balance on token counts) -> every weight matrix crosses HBM
    exactly once, on one core: 1.0 MB/core of bf16 weights instead of
    the 16 MB replicated table.
  * All matmul traffic is cast to bf16 (inputs, weights, outputs). The
    error budget (absmax-relative 2e-2) leaves ~10x margin over bf16's
    ~1e-3.
  * Each core gets NSLOT expert slots; slot j is padded to a uniform
    per-rank token capacity cap[j] (slots sorted by count, so rank
    capacities hug the actual distribution).  One DRAM input tensor per
    core packs [w | xT] per slot in consumption order.

Device side (one static Tile program, identical on all 8 cores):
  Output features live on PSUM partitions, tokens on the free dim, so
  one expert = one PSUM bank and the bias is a per-partition scalar:
    ps[:, 0:C]       = wT00 @ x0 + wT10 @ x1      (i-chunk 0)
    ps[:, 256:256+C] = wT01 @ x0 + wT11 @ x1      (i-chunk 1)
    y0 = max(ps[:, 0:C] + bias0, 0)      (DVE, per-partition bias)
    y1 = relu(ps[:, 256:..] * 1 + bias1) (ACT, runs parallel to DVE)
  No bias matmuls, no weight duplication; 8 slots = 8 PSUM banks, so no
  PSUM reuse waits.  The in-stream rides the single SWDGE (gpsimd)
  queue in need order; outputs leave via the Sync HWDGE ring.  Dummy
  DVE/ACT ops absorb the bias-DMA wait so every matmul and drain
  carries at most one semaphore wait (walrus constraint).

Host side: scatter the per-slot token blocks back to sample order.
Pathological expert skew (>256 tokens per chunk slot or >8 slots per
core) falls back to extra passes of the same program shape.
"""

import os

import numpy as np

import concourse.bacc as bacc
import concourse.bass as bass
import concourse.mybir as mybir
import concourse.tile as tile
from concourse.bass_utils import run_bass_kernel_spmd

try:
    import ml_dtypes

    BF16 = ml_dtypes.bfloat16
except ImportError:  # pragma: no cover
    BF16 = np.dtype("bfloat16")

N_CORES = 8
P = 128
F = 256
CAP_MAX = 256    # tokens per slot (2 i-chunks of <=256 f32 fill one PSUM bank)
NSLOT = 8        # expert slots per core per pass == PSUM banks

# Set by the last kernel() call when KBENCH_TRACE=1 (used by test.py only).
LAST_EXEC_TIME_NS = None
LAST_TRACE = None

_PROGRAM_CACHE = {}


def _build_schedule(idxs: np.ndarray):
    """Assign expert chunks (<=CAP_MAX tokens) to cores, balanced by count.

    Returns (passes, order) where passes is a list of scheduling passes;
    each pass is a list of per-core slot lists [(expert, tok_array), ...]
    sorted by descending token count.
    """
    toks_by_e = [np.nonzero(idxs == e)[0] for e in range(64)]
    chunks = []
    for e, toks in enumerate(toks_by_e):
        for k in range(0, len(toks), CAP_MAX):
            chunks.append((e, toks[k:k + CAP_MAX]))
    chunks.sort(key=lambda c: -len(c[1]))

    npass = max(1, -(-len(chunks) // (N_CORES * NSLOT)))
    cores = [[] for _ in range(N_CORES * npass)]
    load = [0] * (N_CORES * npass)
    for e, toks in chunks:
        cand = min(
            (i for i in range(len(cores)) if len(cores[i]) < NSLOT),
            key=lambda i: load[i],
        )
        cores[cand].append((e, toks))
        load[cand] += len(toks)
    for sl in cores:
        sl.sort(key=lambda c: -len(c[1]))
    return [cores[p * N_CORES:(p + 1) * N_CORES] for p in range(npass)]


def _build_program(caps: tuple):
    nslot = len(caps)
    S = sum(caps)
    xoff = [0]
    for c in caps:
        xoff.append(xoff[-1] + c)

    int8_w = os.environ.get("KINT8", "1") == "1"

    nc = bacc.Bacc(
        "TRN2", target_bir_lowering=False, debug=False, num_devices=N_CORES
    )
    bf16 = mybir.dt.bfloat16
    f32 = mybir.dt.float32
    bias_d = nc.dram_tensor("bias", [P, 2 * nslot], f32, kind="ExternalInput").ap()
    y_d = nc.dram_tensor("y", [P, 2 * S], bf16, kind="ExternalOutput").ap()

    relu = mybir.ActivationFunctionType.Relu
    add = mybir.AluOpType.add
    amax = mybir.AluOpType.max

    # in-stream batches over whole slots: small head batch to prime the
    # pipeline.  The int8 path pays two DMAs (w, x) per batch, so it uses
    # fewer, bigger batches to keep SWDGE descriptor generation (~0.7us per
    # DMA) below the wire time.
    if nslot <= 2:
        groups = [[j] for j in range(nslot)]
    elif int8_w:
        mid = (nslot + 1) // 2
        groups = [[0], list(range(1, mid + 1)), list(range(mid + 1, nslot))]
        groups = [g for g in groups if g]
    else:
        groups = [[0]]
        mid = list(range(1, nslot - 1))
        groups += [mid[i:i + 3] for i in range(0, len(mid), 3)]
        groups += [[nslot - 1]]

    inq = nc.gpsimd
    outq = nc.sync

    with tile.TileContext(nc) as tc:
        with (
            tc.tile_pool(name="const", bufs=1) as const,
            tc.tile_pool(name="inb", bufs=1) as inpool,
            tc.tile_pool(name="yout", bufs=1) as ypool,
            tc.tile_pool(name="ps", bufs=8, space="PSUM") as pspool,
        ):
            bt = const.tile([P, 2 * nslot], f32, tag="bias")
            outq.dma_start(bt[:], bias_d[:])

            tiles = {}
            xtiles = {}
            if int8_w:
                # w ships as int8 (values in [-127,127], exact in bf16) and
                # the SWDGE casts to bf16 in flight; the per-contraction-row
                # quant scales are folded into x on the host.  w and x
                # alternate on the one gpsimd queue in need order, so a slot
                # still needs only a single (later-threshold) wait.
                w_d = nc.dram_tensor(
                    "wt", [P, nslot * 4 * P], mybir.dt.int8, kind="ExternalInput"
                ).ap()
                x_d = nc.dram_tensor(
                    "xt", [P, 2 * S], bf16, kind="ExternalInput"
                ).ap()
                for g, slots in enumerate(groups):
                    lo, hi = slots[0], slots[-1] + 1
                    t = inpool.tile([P, (hi - lo) * 4 * P], bf16, tag=f"w{g}")
                    for j in slots:
                        tiles[j] = (t, (j - lo) * 4 * P)
                    inq.dma_start(t[:], w_d[:, lo * 4 * P:hi * 4 * P])
                    xt = inpool.tile(
                        [P, 2 * (xoff[hi] - xoff[lo])], bf16, tag=f"x{g}"
                    )
                    for j in slots:
                        xtiles[j] = (xt, 2 * (xoff[j] - xoff[lo]))
                    inq.dma_start(xt[:], x_d[:, 2 * xoff[lo]:2 * xoff[hi]])
            else:
                NCOL = nslot * 4 * P + 2 * S
                in_d = nc.dram_tensor(
                    "inp", [P, NCOL], bf16, kind="ExternalInput"
                ).ap()

                def slot_col(j):
                    return j * 4 * P + 2 * xoff[j]

                for g, slots in enumerate(groups):
                    lo, hi = slots[0], slots[-1] + 1
                    a, b = slot_col(lo), slot_col(hi) if hi < nslot else NCOL
                    t = inpool.tile([P, b - a], bf16, tag=f"in{g}")
                    for j in slots:
                        tiles[j] = (t, slot_col(j) - a)
                        xtiles[j] = (t, slot_col(j) - a + 4 * P)
                    inq.dma_start(t[:], in_d[:, a:b])

            def wv(j, c0, c1):
                t, base = tiles[j]
                o = base + (c0 * 2 + c1) * P
                return t[:, o:o + P]

            def xv(j, c0):
                t, base = xtiles[j]
                o = base + c0 * caps[j]
                return t[:, o:o + caps[j]]

            yt0 = ypool.tile([P, S], bf16, tag="y0")
            yt1 = ypool.tile([P, S], bf16, tag="y1")
            scr0 = const.tile([P, 1], f32, tag="scr0")
            scr1 = const.tile([P, 1], f32, tag="scr1")
            # Absorb the bias-DMA wait (and ACT's one-time relu table load)
            # off the critical path so the real drains carry only the PE
            # semaphore wait.
            nc.vector.tensor_scalar(scr0[:], bt[:, 0:1], 0.0, None, add)
            nc.scalar.activation(scr1[:], bt[:, 0:1], relu)

            owave = [0]
            flush_at = {nslot - 1}
            if nslot > 2:
                flush_at.add(nslot - 3)
            for j in range(nslot):
                C = caps[j]
                ps = pspool.tile([P, 2 * F], f32)
                # One accumulation group per PSUM bank: start=True zeroes the
                # WHOLE bank, so only the first matmul opens it and only the
                # last one closes it.
                nc.tensor.matmul(
                    ps[:, 0:C], wv(j, 0, 0), xv(j, 0), start=True, stop=False
                )
                nc.tensor.matmul(
                    ps[:, F:F + C], wv(j, 0, 1), xv(j, 0), start=False, stop=False
                )
                nc.tensor.matmul(
                    ps[:, 0:C], wv(j, 1, 0), xv(j, 1), start=False, stop=False
                )
                nc.tensor.matmul(
                    ps[:, F:F + C], wv(j, 1, 1), xv(j, 1), start=False, stop=True
                )
                o = xoff[j]
                nc.vector.tensor_scalar(
                    yt0[:, o:o + C], ps[:, 0:C], bt[:, 2 * j:2 * j + 1], 0.0,
                    add, amax,
                )
                nc.scalar.activation(
                    yt1[:, o:o + C], ps[:, F:F + C], relu,
                    bias=bt[:, 2 * j + 1:2 * j + 2],
                )
                # Flush outputs in waves; the last wave is just the smallest
                # slot so the post-stream tail stays short.
                if j in flush_at:
                    # yt0 flushes ride sync, yt1 flushes ride the gpsimd
                    # queue (idle once the in-stream is done) so the issue
                    # costs overlap instead of serializing on one ring.
                    lo, hi = owave[0], xoff[j + 1]
                    owave = [hi]
                    outq.dma_start(y_d[:, lo:hi], yt0[:, lo:hi])
                    inq.dma_start(y_d[:, S + lo:S + hi], yt1[:, lo:hi])
    nc.compile()
    return nc


def kernel(x: np.ndarray, idxs: np.ndarray, w: np.ndarray, b: np.ndarray) -> np.ndarray:
    global LAST_EXEC_TIME_NS, LAST_TRACE
    x = np.ascontiguousarray(x, dtype=np.float32)
    w = np.ascontiguousarray(w, dtype=np.float32)
    b = np.ascontiguousarray(b, dtype=np.float32)
    idxs_np = np.asarray(idxs).astype(np.int64)
    B = x.shape[0]

    int8_w = os.environ.get("KINT8", "1") == "1"
    if int8_w:
        # Symmetric int8 per (expert, contraction row) o; scales are folded
        # into x per slot on the host, so the device sees plain bf16 math.
        wscale = np.abs(w).max(axis=1) / 127.0          # (64, 256)
        wscale = np.maximum(wscale, 1e-30)
        wq = np.round(w / wscale[:, None, :]).clip(-127, 127).astype(np.int8)
        wblk = np.ascontiguousarray(
            wq.reshape(64, 2, P, 2, P)     # (e, c1, m, c0, p)
            .transpose(0, 4, 3, 1, 2)      # (e, p, c0, c1, m)
            .reshape(64, P, 4 * P)
        )
    else:
        x16 = x.astype(BF16)
        # Per-expert weight blocks in PE layout:
        # wblk[e, p, (c0*2+c1)*128 + m] = w[e, c1*128+m, c0*128+p]
        wblk = np.ascontiguousarray(
            w.reshape(64, 2, P, 2, P)          # (e, c1, m, c0, p)
            .transpose(0, 4, 3, 1, 2)          # (e, p, c0, c1, m)
            .reshape(64, P, 4 * P)
            .astype(BF16)
        )

    passes = _build_schedule(idxs_np)
    trace = bool(os.environ.get("KBENCH_TRACE"))
    y = np.empty((B, F), dtype=np.float32)

    for cores in passes:
        nslot = max(1, max(len(sl) for sl in cores))
        caps = tuple(
            max(4, -4 * (-max(
                (len(sl[j][1]) if j < len(sl) else 0) for sl in cores
            ) // 4))
            for j in range(nslot)
        )
        S = sum(caps)
        xoff = np.concatenate([[0], np.cumsum(caps)]).astype(int)
        NCOL = nslot * 4 * P + 2 * S

        key = caps
        if key not in _PROGRAM_CACHE:
            _PROGRAM_CACHE[key] = _build_program(caps)
        nc = _PROGRAM_CACHE[key]

        in_maps = []
        for sl in cores:
            bias = np.zeros((P, 2 * nslot), dtype=np.float32)
            if int8_w:
                wt = np.zeros((P, nslot * 4 * P), dtype=np.int8)
                xt_full = np.zeros((P, 2 * S), dtype=BF16)
            else:
                inp = np.zeros((P, NCOL), dtype=BF16)
            for j, (e, toks) in enumerate(sl):
                n = len(toks)
                if int8_w:
                    wt[:, j * 4 * P:(j + 1) * 4 * P] = wblk[e]
                    xs = (x[toks] * wscale[e]).astype(BF16)
                    xt = xs.T.reshape(2, P, n).transpose(1, 0, 2)
                    xcols = xt_full[:, 2 * xoff[j]:2 * xoff[j + 1]].reshape(
                        P, 2, caps[j]
                    )
                else:
                    col = j * 4 * P + 2 * xoff[j]
                    inp[:, col:col + 4 * P] = wblk[e]
                    # xT[p, c0, t] = x[tok_t, c0*128 + p]
                    xt = x16[toks].T.reshape(2, P, n).transpose(1, 0, 2)
                    xcols = inp[:, col + 4 * P:col + 4 * P + 2 * caps[j]].reshape(
                        P, 2, caps[j]
                    )
                xcols[:, :, :n] = xt
                bias[:, 2 * j] = b[e, 0:P]
                bias[:, 2 * j + 1] = b[e, P:2 * P]
            if int8_w:
                in_maps.append({"wt": wt, "xt": xt_full, "bias": bias})
            else:
                in_maps.append({"inp": inp, "bias": bias})

        res = run_bass_kernel_spmd(
            nc, in_maps, core_ids=list(range(N_CORES)), trace=trace
        )
        LAST_EXEC_TIME_NS = res.exec_time_ns
        LAST_TRACE = res.instructions_and_trace

        for c, sl in enumerate(cores):
            yc = np.asarray(res.results[c]["y"]).reshape(P, 2, S)
            for j, (e, toks) in enumerate(sl):
                n = len(toks)
                o = xoff[j]
                y[toks] = (
                    yc[:, :, o:o + n].transpose(2, 1, 0).reshape(n, F)
                )
    return y
